# revision 1
# baseline (speedup 1.0000x reference)
"""Trainium2 Bass kernel for nn_Block_63591285784865 (attention + dense-MoE block).

Sharding: pure token-parallel across 8 NeuronCores, no collectives.
Core c handles batch b = c//2, token half = c%2 (512 query tokens).
Each core computes LN1 + K/V for the full 1024-token batch row (cheap),
attention/proj/LN2/router/all-8-experts for its own 512 tokens.

On-device layout is feature-major ("transposed"): activations live as
[C(partitions), tokens(free)] so matmuls chain without transposes.
LayerNorm gains/biases are folded into the consuming weights on the host.
Matmuls run in bf16 with fp32 PSUM accumulation; softmax/LN/router in fp32.
"""

import numpy as np
import ml_dtypes

BF16 = ml_dtypes.bfloat16

B, T, C = 4, 1024, 512
H, HD = 8, 64
NE, K = 8, 2
FF = 4 * C
S = 512          # tokens per core
NCORES = 8
EPS = 1e-5
NEG = -1e9

_CACHE = {}


# ---------------------------------------------------------------- tile patch
def _apply_tile_patch():
    """walrus in this container rejects >2 sem waits on one instruction; the
    TileContext exit drain accumulates one wait per proc.  Split it."""
    from concourse.tile import ScopedClock, TileContext
    from concourse.tile_sem_assignment import VectorClock

    if getattr(TileContext, "_drain_patched", False):
        return

    def _drain_and_barrier_split(self, tick_clock, wait_clock):
        nc = self.nc
        gc = tick_clock.global_clock
        n = 27
        for p in range(n):
            if gc[p] <= 0:
                continue
            vals = [gc[q] if q == p else 0 for q in range(n)]
            d = nc.sync.drain()
            wait_clock.add_sem_waits(d.ins, ScopedClock({None: VectorClock(vals)}))
        nc.all_engine_barrier()
        popped = nc._tile_sem_poison_stack.pop()
        assert popped is self._sem_poison
        nc.clear_and_free_semaphores(list(self.sems.allocated().values()))
        nc.all_engine_barrier()

    TileContext._drain_and_barrier = _drain_and_barrier_split
    TileContext._drain_patched = True


def _split_sync_waits(nc, mybir, limit=2):
    """This walrus build rejects instructions carrying more than a couple of
    semaphore waits (1 for DMA/drain); hoist the excess onto preceding
    same-engine NoOps."""
    nid = [0]
    tight = (mybir.InstDMACopy, mybir.InstDMA, mybir.InstDrain,
             mybir.InstDmaTransposeAnt, mybir.InstTensorLoad,
             mybir.InstTensorSave)
    for f in nc.m.functions:
        for bb in f.blocks:
            insts = bb.instructions
            out = []
            for inst in insts:
                limit = 1
                si = inst.sync_info
                waits = list(si.on_wait) if si and si.on_wait else []
                if len(waits) > limit:
                    keep = waits[-limit:]
                    extra = waits[:-limit]
                    for i in range(0, len(extra), limit):
                        nid[0] += 1
                        nop = mybir.InstNoOp(
                            name=f"I-waitsplit-{nid[0]}",
                            engine=inst.engine,
                            ins=[], outs=[],
                            sync_info=mybir.SyncInfo(
                                on_wait=extra[i:i + limit], on_update=[]),
                        )
                        nc.register_instruction(nop, overwrite=True)
                        out.append(nop)
                    inst.sync_info = mybir.SyncInfo(
                        on_wait=keep, on_update=list(si.on_update or []))
                out.append(inst)
            bb.instructions = out


# ---------------------------------------------------------------- program
def build_program():
    import concourse.bass as bass
    import concourse.mybir as mybir
    import concourse.tile as tile

    _apply_tile_patch()

    f32 = mybir.dt.float32
    bf16 = mybir.dt.bfloat16
    Alu = mybir.AluOpType
    Act = mybir.ActivationFunctionType
    AX = mybir.AxisListType.X

    nc = bass.Bass()
    dp = nc.declare_dram_parameter
    # per-core inputs
    xbt_d = dp("xbt", [C, T], f32, isOutput=False)       # x[b].T, my tokens first
    kvb_d = dp("kvb", [128, 8], f32, isOutput=False)     # per-kv-chunk additive bias
    # shared inputs
    wq_d = dp("wq", [C, H * HD], f32, isOutput=False)
    wk_d = dp("wk", [C, H * HD], f32, isOutput=False)
    wv_d = dp("wv", [C, H * HD], f32, isOutput=False)
    wp_d = dp("wp", [H * HD, C], f32, isOutput=False)
    qb_d = dp("qb", [128, 4], f32, isOutput=False)
    kb_d = dp("kb", [128, 4], f32, isOutput=False)
    vbr_d = dp("vbr", [1, H * HD], f32, isOutput=False)
    bp_d = dp("bp", [128, 4], f32, isOutput=False)
    wr_d = dp("wr", [C, NE], f32, isOutput=False)
    rbr_d = dp("rbr", [1, NE], f32, isOutput=False)
    w1_d = dp("w1", [NE, C, FF], bf16, isOutput=False)
    b1s_d = dp("b1s", [128, NE * 16], f32, isOutput=False)
    w2_d = dp("w2", [NE, FF, C], bf16, isOutput=False)
    b2_d = dp("b2", [NE, C], f32, isOutput=False)
    sel_d = dp("sel", [NE, NE * 128], f32, isOutput=False)
    out_d = dp("out", [C, S], f32, isOutput=True)

    from concourse.masks import make_identity

    with tile.TileContext(nc) as tc:
        f32r = mybir.dt.float32r

        with tc.tile_pool(name="persist", bufs=1) as pp:
            # ---------------- persistent constants
            ones_col_f = pp.tile([128, 1], f32, name="ones_col_f", tag="ones_col_f")
            nc.vector.memset(ones_col_f[:], 1.0)
            ones_col_r = pp.tile([128, 1], f32r, name="ones_col_r", tag="ones_col_r")
            nc.vector.tensor_copy(ones_col_r[:], ones_col_f[:])
            ones128 = pp.tile([128, 128], f32, name="ones128", tag="ones128")
            nc.vector.memset(ones128[:], 1.0)
            ones_row = ones128[0:1, :]
            ones_row_m = ones128[32:33, :]
            ident = pp.tile([128, 128], f32, name="ident", tag="ident")
            make_identity(nc, ident[:])
            eps_t = pp.tile([1, 1], f32, name="eps_t", tag="eps_t")
            nc.vector.memset(eps_t[:], EPS)
            selc32 = pp.tile([32, NE * 128], f32, name="selc32", tag="selc32")
            nc.sync.dma_start(selc32[0:NE, :], sel_d[:])
            b1s = pp.tile([128, NE * 16], f32, name="b1s", tag="b1s")
            nc.sync.dma_start(b1s[:], b1s_d[:])
            b2sb32 = pp.tile([32, C], f32, name="b2sb32", tag="b2sb32")
            nc.sync.dma_start(b2sb32[0:NE, :], b2_d[:])
            rT32 = pp.tile([32, S], f32, name="rT32", tag="rT32")
            rT = rT32[0:NE, :]
            x2T = [pp.tile([128, S], f32, name=f"x2T{ct}", tag=f"x2T{ct}")
                   for ct in range(4)]
            h2T = [pp.tile([128, S], bf16, name=f"h2T{ci}", tag=f"h2T{ci}")
                   for ci in range(4)]

            with tc.tile_pool(name="attn_era", bufs=1) as ae:
                # ---------------- attention-era constants
                kvb = ae.tile([128, 8], f32, name="kvb", tag="kvb")
                nc.sync.dma_start(kvb[:], kvb_d[:])
                qb = ae.tile([128, 4], f32, name="qb", tag="qb")
                nc.sync.dma_start(qb[:], qb_d[:])
                kb = ae.tile([128, 4], f32, name="kb", tag="kb")
                nc.sync.dma_start(kb[:], kb_d[:])
                bp = ae.tile([128, 4], f32, name="bp", tag="bp")
                nc.sync.dma_start(bp[:], bp_d[:])
                vbr32 = ae.tile([32, H * HD], f32, name="vbr32", tag="vbr32")
                nc.sync.dma_start(vbr32[0:1, :], vbr_d[:])
                rbr32 = ae.tile([32, NE], f32, name="rbr32", tag="rbr32")
                nc.sync.dma_start(rbr32[0:1, :], rbr_d[:])
                mask4 = []
                for j in range(4):
                    m = ae.tile([128, S], f32, name=f"mask{j}", tag=f"mask{j}")
                    nc.gpsimd.memset(m[:], 0.0)
                    nc.gpsimd.affine_select(
                        out=m[:], in_=m[:],
                        compare_op=Alu.is_ge, fill=NEG,
                        base=-128 * j, channel_multiplier=-1,
                        pattern=[[1, S]],
                    )
                    mask4.append(m)
                wq = [ae.tile([128, H * HD], f32r, name=f"wq{ci}", tag=f"wq{ci}")
                      for ci in range(4)]
                wk = [ae.tile([128, H * HD], f32r, name=f"wk{ci}", tag=f"wk{ci}")
                      for ci in range(4)]
                wv = [ae.tile([128, H * HD], f32r, name=f"wv{ci}", tag=f"wv{ci}")
                      for ci in range(4)]
                wp = [ae.tile([128, C], f32r, name=f"wp{ci}", tag=f"wp{ci}")
                      for ci in range(4)]
                wr = [ae.tile([128, NE], f32r, name=f"wr{ci}", tag=f"wr{ci}")
                      for ci in range(4)]
                with tc.tile_pool(name="wstage", bufs=2) as wst:
                    for ci in range(4):
                        cs = slice(128 * ci, 128 * (ci + 1))
                        for wdst, wsrc in ((wq, wq_d), (wk, wk_d), (wv, wv_d),
                                           (wp, wp_d), (wr, wr_d)):
                            stg = wst.tile([128, wdst[ci].shape[1]], f32,
                                           name="stg", tag="stg")
                            nc.sync.dma_start(stg[:], wsrc[cs, :])
                            nc.vector.tensor_copy(wdst[ci][:], stg[:])
                hT = [ae.tile([128, T], f32r, name=f"hT{ci}", tag=f"hT{ci}")
                      for ci in range(4)]
                qTs = [ae.tile([128, S], f32r, name=f"qT{ft}", tag=f"qT{ft}")
                       for ft in range(4)]
                kTs = [ae.tile([128, T], f32r, name=f"kT{ft}", tag=f"kT{ft}")
                       for ft in range(4)]
                vS = [ae.tile([128, H * HD], f32r, name=f"v{j}", tag=f"v{j}")
                      for j in range(8)]
                vb_b = ae.tile([128, H * HD], f32, name="vb_b", tag="vb_b")
                attnT2 = [ae.tile([128, S], f32r, name=f"attnT{ft}", tag=f"attnT{ft}")
                          for ft in range(4)]
                h2Tf = [ae.tile([128, S], f32r, name=f"h2Tf{ci}", tag=f"h2Tf{ci}")
                        for ci in range(4)]
                # LN row scratch; quantities at partition starts 0/32/64/96
                rows1 = ae.tile([128, 2 * T], f32, name="rows1", tag="rows1")
                rows2 = ae.tile([128, 2 * S], f32, name="rows2", tag="rows2")

                # ---------------- LN1 (pure standardize; affine folded into W)
                with (
                    tc.tile_pool(name="xbt_era", bufs=1) as xe,
                    tc.tile_pool(name="ln1sb", bufs=1) as lsb,
                    tc.tile_pool(name="ln1ps", bufs=2, space="PSUM") as lps,
                ):
                    xbT = [xe.tile([128, T], f32, name=f"xbT{ci}", tag=f"xbT{ci}")
                           for ci in range(4)]
                    for ci in range(4):
                        nc.sync.dma_start(xbT[ci][:],
                                          xbt_d[128 * ci:128 * (ci + 1), :])
                    for th in range(2):
                        ts_ = slice(512 * th, 512 * (th + 1))
                        ts2 = slice(T + 512 * th, T + 512 * (th + 1))
                        mu = rows1[64:65, ts_]
                        musq = rows1[96:97, ts_]
                        var = rows1[0:1, ts2]
                        sd = rows1[32:33, ts2]
                        psum = lps.tile([1, 512], f32, name="psum", tag="s")
                        psq = lps.tile([1, 512], f32, name="psq", tag="sq")
                        for ci in range(4):
                            nc.tensor.matmul(psum[:], ones_col_f[:],
                                             xbT[ci][:, ts_],
                                             start=(ci == 0), stop=(ci == 3))
                        for ci in range(4):
                            xq_t = lsb.tile([128, 512], f32, name="xq_t", tag="xsq")
                            nc.scalar.square(xq_t[:], xbT[ci][:, ts_])
                            nc.tensor.matmul(psq[:], ones_col_f[:], xq_t[:],
                                             start=(ci == 0), stop=(ci == 3))
                        nc.vector.tensor_scalar_mul(mu, psum[:], 1.0 / C)
                        nc.vector.tensor_tensor(musq, mu, mu, Alu.mult)
                        nc.vector.scalar_tensor_tensor(var, psq[:], 1.0 / C,
                                                       musq, Alu.mult, Alu.subtract)
                        nc.scalar.activation(sd, var, Act.Sqrt, bias=eps_t[:])
                        nc.vector.reciprocal(rows1[0:1, ts_], sd)
                        nc.vector.scalar_tensor_tensor(rows1[32:33, ts_], psum[:],
                                                       1.0 / C, rows1[0:1, ts_],
                                                       Alu.mult, Alu.mult)
                        # broadcast across partitions via ones-matmul, keep in PSUM
                        prs = lps.tile([128, 512], f32, name="prs", tag="prs")
                        nc.tensor.matmul(prs[:], ones_row, rows1[0:1, ts_],
                                         start=True, stop=True)
                        pms = lps.tile([128, 512], f32, name="pms", tag="pms")
                        nc.tensor.matmul(pms[:], ones_row_m,
                                         rows1[32:33, ts_],
                                         start=True, stop=True)
                        for ci in range(4):
                            tmp = lsb.tile([128, 512], f32, name="tmp", tag="nrm")
                            nc.vector.tensor_tensor(tmp[:], xbT[ci][:, ts_],
                                                    prs[:], Alu.mult)
                            nc.vector.tensor_tensor(hT[ci][:, ts_], tmp[:],
                                                    pms[:], Alu.subtract)

                # ---------------- QKV
                with tc.tile_pool(name="qkvps", bufs=2, space="PSUM") as qps:
                    pb = qps.tile([128, H * HD], f32, name="pb", tag="vbb", bufs=1)
                    nc.tensor.matmul(pb[:], ones_row, vbr32[0:1, :],
                                     start=True, stop=True)
                    nc.scalar.copy(vb_b[:], pb[:])
                    for ft in range(4):
                        fs = slice(128 * ft, 128 * (ft + 1))
                        pq = qps.tile([128, S], f32, name="pq", tag="pq")
                        for ci in range(4):
                            nc.tensor.matmul(pq[:], wq[ci][:, fs],
                                             hT[ci][:, 0:S],
                                             start=(ci == 0), stop=(ci == 3))
                        nc.scalar.activation(qTs[ft][:], pq[:], Act.Identity,
                                             bias=qb[:, ft:ft + 1], scale=1.0)
                        for th in range(2):
                            ts_ = slice(512 * th, 512 * (th + 1))
                            pk = qps.tile([128, 512], f32, name="pk", tag="pk")
                            for ci in range(4):
                                nc.tensor.matmul(pk[:], wk[ci][:, fs],
                                                 hT[ci][:, ts_],
                                                 start=(ci == 0), stop=(ci == 3))
                            nc.scalar.activation(kTs[ft][:, ts_], pk[:], Act.Identity,
                                                 bias=kb[:, ft:ft + 1], scale=1.0)
                    for j in range(8):
                        js = slice(128 * j, 128 * (j + 1))
                        pv = qps.tile([128, H * HD], f32, name="pv", tag="pv")
                        for ci in range(4):
                            nc.tensor.matmul(pv[:], hT[ci][:, js], wv[ci][:],
                                             start=(ci == 0), stop=(ci == 3))
                        nc.vector.tensor_tensor(vS[j][:], pv[:], vb_b[:], Alu.add)

                # ---------------- attention, two heads in flight
                with (
                    tc.tile_pool(name="attnps", bufs=2, space="PSUM") as aps,
                    tc.tile_pool(name="attnsb", bufs=3) as asb,
                ):
                    for hp in range(4):
                        ft = hp
                        pav = [aps.tile([64, S], f32, name=f"pav{sub}",
                                        tag=f"pav{sub}", bufs=1) for sub in range(2)]
                        pd = [aps.tile([1, S], f32, name=f"pd{sub}",
                                       tag=f"pd{sub}", bufs=1) for sub in range(2)]
                        for j in range(8):
                            js = slice(128 * j, 128 * (j + 1))
                            pt2 = []
                            for sub in range(2):
                                h = 2 * hp + sub
                                rs = slice(64 * sub, 64 * (sub + 1))
                                ps = aps.tile([128, S], f32, name="ps", tag="ps",
                                              bufs=3)
                                nc.tensor.matmul(ps[:], kTs[ft][rs, js],
                                                 qTs[ft][rs, :],
                                                 start=True, stop=True)
                                pt = asb.tile([128, S], f32r, name="pt", tag="pt",
                                              bufs=4)
                                if j < 4:
                                    tmp = asb.tile([128, S], f32, name="tmp2",
                                                   tag="ptmp")
                                    nc.vector.tensor_tensor(tmp[:], ps[:],
                                                            mask4[j][:], Alu.add)
                                    nc.scalar.activation(pt[:], tmp[:], Act.Exp,
                                                         bias=kvb[:, j:j + 1],
                                                         scale=1.0)
                                else:
                                    nc.scalar.activation(pt[:], ps[:], Act.Exp,
                                                         bias=kvb[:, j:j + 1],
                                                         scale=1.0)
                                pt2.append(pt)
                            for sub in range(2):
                                h = 2 * hp + sub
                                nc.tensor.matmul(pd[sub][:], ones_col_r[:],
                                                 pt2[sub][:],
                                                 start=(j == 0), stop=(j == 7),
                                                 skip_group_check=True)
                                nc.tensor.matmul(pav[sub][:],
                                                 vS[j][:, 64 * h:64 * (h + 1)],
                                                 pt2[sub][:],
                                                 start=(j == 0), stop=(j == 7),
                                                 skip_group_check=True)
                        for sub in range(2):
                            rs = slice(64 * sub, 64 * (sub + 1))
                            rd32 = asb.tile([32, S], f32, name="rd32", tag="rd")
                            rd = rd32[0:1, :]
                            nc.vector.reciprocal(rd, pd[sub][:])
                            prb = aps.tile([128, S], f32, name="prb", tag="prb",
                                           bufs=1)
                            nc.tensor.matmul(prb[:], ones_row, rd,
                                             start=True, stop=True)
                            rb = asb.tile([128, S], f32, name="rb", tag="rb")
                            nc.scalar.copy(rb[:], prb[:])
                            nc.vector.tensor_tensor(attnT2[ft][rs, :], pav[sub][:],
                                                    rb[0:64, :], Alu.mult)

                # ---------------- proj + residual -> x2T
                with (
                    tc.tile_pool(name="projps", bufs=4, space="PSUM") as pps,
                    tc.tile_pool(name="projsb", bufs=2) as psb,
                ):
                    for ct in range(4):
                        cs = slice(128 * ct, 128 * (ct + 1))
                        px = pps.tile([128, S], f32, name="px", tag="px")
                        for hd in range(4):
                            nc.tensor.matmul(px[:], wp[hd][:, cs],
                                             attnT2[hd][:],
                                             start=(hd == 0), stop=(hd == 3))
                        xq_r = psb.tile([128, S], f32, name="xq_r", tag="xq_r")
                        nc.sync.dma_start(xq_r[:], xbt_d[cs, 0:S])
                        nc.vector.scalar_tensor_tensor(x2T[ct][:], px[:],
                                                       bp[:, ct:ct + 1], xq_r[:],
                                                       Alu.add, Alu.add)

                # ---------------- LN2 -> h2Tf (f32) + h2T (bf16)
                with (
                    tc.tile_pool(name="ln2sb", bufs=1) as lsb2,
                    tc.tile_pool(name="ln2ps", bufs=2, space="PSUM") as lps2,
                ):
                    mu = rows2[64:65, 0:S]
                    musq = rows2[96:97, 0:S]
                    var = rows2[0:1, S:2 * S]
                    sd = rows2[32:33, S:2 * S]
                    psum = lps2.tile([1, S], f32, name="psum2", tag="s", bufs=1)
                    psq = lps2.tile([1, S], f32, name="psq2", tag="sq", bufs=1)
                    for ci in range(4):
                        nc.tensor.matmul(psum[:], ones_col_f[:], x2T[ci][:],
                                         start=(ci == 0), stop=(ci == 3))
                    for ci in range(4):
                        xq_t = lsb2.tile([128, S], f32, name="xq_t2", tag="xsq2")
                        nc.scalar.square(xq_t[:], x2T[ci][:])
                        nc.tensor.matmul(psq[:], ones_col_f[:], xq_t[:],
                                         start=(ci == 0), stop=(ci == 3))
                    nc.vector.tensor_scalar_mul(mu, psum[:], 1.0 / C)
                    nc.vector.tensor_tensor(musq, mu, mu, Alu.mult)
                    nc.vector.scalar_tensor_tensor(var, psq[:], 1.0 / C, musq,
                                                   Alu.mult, Alu.subtract)
                    nc.scalar.activation(sd, var, Act.Sqrt, bias=eps_t[:])
                    nc.vector.reciprocal(rows2[0:1, 0:S], sd)
                    nc.vector.scalar_tensor_tensor(rows2[32:33, 0:S], psum[:],
                                                   1.0 / C, rows2[0:1, 0:S],
                                                   Alu.mult, Alu.mult)
                    prs = lps2.tile([128, S], f32, name="prs2", tag="prs", bufs=1)
                    nc.tensor.matmul(prs[:], ones_row, rows2[0:1, 0:S],
                                     start=True, stop=True)
                    pms = lps2.tile([128, S], f32, name="pms2", tag="pms", bufs=1)
                    nc.tensor.matmul(pms[:], ones_row_m, rows2[32:33, 0:S],
                                     start=True, stop=True)
                    for ci in range(4):
                        tmp = lsb2.tile([128, S], f32, name="tmp3", tag="nrm2")
                        nc.vector.tensor_tensor(tmp[:], x2T[ci][:], prs[:], Alu.mult)
                        nc.vector.tensor_tensor(h2Tf[ci][:], tmp[:], pms[:],
                                                Alu.subtract)
                        nc.vector.tensor_tensor(h2T[ci][:], tmp[:], pms[:],
                                                Alu.subtract)

                # ---------------- router
                with (
                    tc.tile_pool(name="rps", bufs=2, space="PSUM") as rps,
                    tc.tile_pool(name="rsb", bufs=2) as rsb,
                ):
                    prbb = rps.tile([128, NE], f32, name="prbb", tag="rbb", bufs=1)
                    nc.tensor.matmul(prbb[:], ones_row, rbr32[0:1, :],
                                     start=True, stop=True)
                    rbb = rsb.tile([128, NE], f32, name="rbb", tag="rbbs", bufs=1)
                    nc.scalar.copy(rbb[:], prbb[:])
                    for qc in range(4):
                        qs = slice(128 * qc, 128 * (qc + 1))
                        psc = rps.tile([128, NE], f32, name="psc", tag="psc")
                        for ci in range(4):
                            nc.tensor.matmul(psc[:], h2Tf[ci][:, qs],
                                             wr[ci][:],
                                             start=(ci == 0), stop=(ci == 3))
                        sc = rsb.tile([128, NE], f32, name="sc", tag="sc")
                        nc.vector.tensor_tensor(sc[:], psc[:], rbb[:], Alu.add)
                        m1 = rsb.tile([128, 1], f32, name="m1", tag="m1")
                        nc.vector.reduce_max(m1[:], sc[:], axis=AX)
                        eq = rsb.tile([128, NE], f32, name="eq", tag="eq")
                        nc.vector.tensor_scalar(eq[:], sc[:], m1[:], None,
                                                Alu.is_equal)
                        sm = rsb.tile([128, NE], f32, name="sm", tag="sm")
                        nc.vector.scalar_tensor_tensor(sm[:], eq[:], NEG, sc[:],
                                                       Alu.mult, Alu.add)
                        m2 = rsb.tile([128, 1], f32, name="m2", tag="m2")
                        nc.vector.reduce_max(m2[:], sm[:], axis=AX)
                        ge = rsb.tile([128, NE], f32, name="ge", tag="ge")
                        nc.vector.tensor_scalar(ge[:], sc[:], m2[:], None, Alu.is_ge)
                        msk = rsb.tile([128, NE], f32, name="msk", tag="msk")
                        nc.vector.tensor_tensor(msk[:], sc[:], ge[:], Alu.mult)
                        ex = rsb.tile([128, NE], f32, name="ex", tag="ex")
                        nc.scalar.activation(ex[:], msk[:], Act.Exp)
                        dsum = rsb.tile([128, 1], f32, name="dsum", tag="dsum")
                        nc.vector.reduce_sum(dsum[:], ex[:], axis=AX)
                        rec = rsb.tile([128, 1], f32, name="rec", tag="rec")
                        nc.vector.reciprocal(rec[:], dsum[:])
                        rr = rsb.tile([128, NE], f32, name="rr", tag="rr")
                        nc.vector.tensor_scalar_mul(rr[:], ex[:], rec[:])
                        ptr = rps.tile([NE, 128], f32, name="ptr", tag="ptr")
                        nc.tensor.transpose(ptr[:], rr[:], ident[:])
                        nc.scalar.copy(rT32[0:NE, qs], ptr[:])

            # ---------------- MoE: dense all-expert compute, PSUM-resident acc
            with (
                tc.tile_pool(name="w1pool", bufs=2) as w1p,
                tc.tile_pool(name="w2pool", bufs=2) as w2p,
                tc.tile_pool(name="accps", bufs=1, space="PSUM") as accp,
                tc.tile_pool(name="moeps", bufs=2, space="PSUM") as mps,
                tc.tile_pool(name="moesb", bufs=3) as msb,
            ):
                pacc = [accp.tile([128, S], f32, name=f"pacc{ct}", tag=f"acc{ct}")
                        for ct in range(4)]
                for ct in range(4):
                    nc.tensor.matmul(pacc[ct][:],
                                     b2sb32[0:NE, 128 * ct:128 * (ct + 1)],
                                     rT, start=True, stop=False,
                                     skip_group_check=True)
                w1ts, w2ts, prbes = {}, {}, {}
                for e in range(NE):
                    w1ts[e] = [w1p.tile([128, FF], bf16, name=f"w1t{ci}",
                                        tag=f"w1_{ci}") for ci in range(4)]
                    for ci in range(4):
                        nc.sync.dma_start(w1ts[e][ci][:],
                                          w1_d[e, 128 * ci:128 * (ci + 1), :])
                    w2ts[e] = [w2p.tile([128, C], bf16, name=f"w2t{ff}",
                                        tag=f"w2_{ff}") for ff in range(16)]
                    for ff in range(16):
                        nc.sync.dma_start(w2ts[e][ff][:],
                                          w2_d[e, 128 * ff:128 * (ff + 1), :])
                # flat pipelined loop: W1 of step s+1 issues before W2 of s
                pending = None
                for e in range(NE):
                    for ff in range(16):
                        if ff == 0:
                            prbe = mps.tile([128, S], f32, name="prbe", tag="prbe")
                            nc.tensor.matmul(prbe[:],
                                             selc32[0:NE, 128 * e:128 * (e + 1)],
                                             rT, start=True, stop=True)
                            prbes[e] = prbe
                        fs = slice(128 * ff, 128 * (ff + 1))
                        ph = mps.tile([128, S], f32, name="ph", tag="ph")
                        for ci in range(4):
                            nc.tensor.matmul(ph[:], w1ts[e][ci][:, fs], h2T[ci][:],
                                             start=(ci == 0), stop=(ci == 3))
                        if pending is not None:
                            phids, pw2, pe_, pff = pending
                            for ct in range(4):
                                nc.tensor.matmul(pacc[ct][:],
                                                 pw2[:, 128 * ct:128 * (ct + 1)],
                                                 phids[:], start=False, stop=False,
                                                 skip_group_check=True)
                        hidf = msb.tile([128, S], f32, name="hidf", tag="hidf")
                        nc.scalar.activation(hidf[:], ph[:], Act.Relu,
                                             bias=b1s[:, 16 * e + ff:16 * e + ff + 1],
                                             scale=1.0)
                        hids = msb.tile([128, S], bf16, name="hids", tag="hids")
                        nc.vector.tensor_tensor(hids[:], hidf[:], prbes[e][:],
                                                Alu.mult)
                        pending = (hids, w2ts[e][ff], e, ff)
                phids, pw2, pe_, pff = pending
                for ct in range(4):
                    nc.tensor.matmul(pacc[ct][:],
                                     pw2[:, 128 * ct:128 * (ct + 1)],
                                     phids[:], start=False, stop=True,
                                     skip_group_check=True)
                for ct in range(4):
                    ot = msb.tile([128, S], f32, name="ot", tag="ot")
                    nc.vector.tensor_tensor(ot[:], pacc[ct][:], x2T[ct][:], Alu.add)
                    nc.sync.dma_start(out_d[128 * ct:128 * (ct + 1), :], ot[:])

    _split_sync_waits(nc, mybir)
    return nc


# ---------------------------------------------------------------- host side
def prep_inputs(inputs):
    """Returns list of 8 per-core input dicts."""
    x = np.asarray(inputs["x"], np.float32)
    ln1_g = np.asarray(inputs["ln1_g"], np.float32)
    ln1_b = np.asarray(inputs["ln1_b"], np.float32)
    ln2_g = np.asarray(inputs["ln2_g"], np.float32)
    ln2_b = np.asarray(inputs["ln2_b"], np.float32)
    Wq = np.asarray(inputs["Wq"], np.float32)
    Wk = np.asarray(inputs["Wk"], np.float32)
    Wv = np.asarray(inputs["Wv"], np.float32)
    Wp = np.asarray(inputs["Wp"], np.float32)
    bp = np.asarray(inputs["bp"], np.float32)
    Wr = np.asarray(inputs["Wr"], np.float32)
    br = np.asarray(inputs["br"], np.float32)
    W1 = np.asarray(inputs["W1"], np.float32)
    b1 = np.asarray(inputs["b1"], np.float32)
    W2 = np.asarray(inputs["W2"], np.float32)
    b2 = np.asarray(inputs["b2"], np.float32)

    WqT2 = Wq.transpose(1, 0, 2).reshape(C, H * HD)
    WkT2 = Wk.transpose(1, 0, 2).reshape(C, H * HD)
    WvT2 = Wv.transpose(1, 0, 2).reshape(C, H * HD)
    isq = 1.0 / np.sqrt(HD).astype(np.float32)

    wq = (ln1_g[:, None] * WqT2 * isq).astype(np.float32)
    qb = ((ln1_b @ WqT2) * isq).astype(np.float32)
    wk = (ln1_g[:, None] * WkT2).astype(np.float32)
    kb = (ln1_b @ WkT2).astype(np.float32)
    wv = (ln1_g[:, None] * WvT2).astype(np.float32)
    vb = (ln1_b @ WvT2).astype(np.float32)
    wr = (ln2_g[:, None] * Wr).astype(np.float32)
    rb = (br + ln2_b @ Wr).astype(np.float32)
    w1 = (ln2_g[None, :, None] * W1).astype(BF16)
    b1e = (b1 + np.einsum("c,ecf->ef", ln2_b, W1)).astype(np.float32)
    b1s = b1e.reshape(NE, 16, 128).transpose(2, 0, 1).reshape(128, NE * 16)
    b1s = np.ascontiguousarray(b1s, np.float32)

    sel = np.zeros((NE, NE * 128), np.float32)
    for e in range(NE):
        sel[e, 128 * e:128 * (e + 1)] = 1.0

    shared = {
        "wq": wq, "wk": wk, "wv": wv,
        "wp": Wp.astype(np.float32),
        "qb": np.ascontiguousarray(qb.reshape(4, 128).T),
        "kb": np.ascontiguousarray(kb.reshape(4, 128).T),
        "vbr": vb[None, :],
        "bp": np.ascontiguousarray(bp.reshape(4, 128).T),
        "wr": wr, "rbr": rb[None, :],
        "w1": w1, "b1s": b1s,
        "w2": W2.astype(BF16),
        "b2": b2.astype(np.float32),
        "sel": sel,
    }

    in_maps = []
    for c in range(NCORES):
        b, half = divmod(c, 2)
        perm = np.r_[half * S:(half + 1) * S, (1 - half) * S:(2 - half) * S]
        xbt = np.ascontiguousarray(x[b][perm].T)  # [C, T], my tokens first
        kvb = np.zeros((128, 8), np.float32)
        if half == 0:
            kvb[:, 4:] = NEG
        m = dict(shared)
        m["xbt"] = xbt
        m["kvb"] = kvb
        in_maps.append(m)
    return in_maps


def gather_outputs(results):
    out = np.empty((B, T, C), np.float32)
    for c in range(NCORES):
        b, half = divmod(c, 2)
        out[b, half * S:(half + 1) * S, :] = results[c]["out"].T
    return out


def kernel(**inputs):
    from concourse.bass_utils import run_bass_kernel_spmd

    if "nc" not in _CACHE:
        _CACHE["nc"] = build_program()
    nc = _CACHE["nc"]
    in_maps = prep_inputs(inputs)
    res = run_bass_kernel_spmd(nc, in_maps, list(range(NCORES)))
    return gather_outputs(res.results)

